# revision 15
# baseline (speedup 1.0000x reference)
"""Trainium2 Bass kernel for the 3-block invertible coupling flow (RealNVP-style).

Computation (per sample row of x = [u1(256) | u2(256) | t(1)]):
    for j in 3 blocks:
        v1 = u1 * exp(mlp_s2(u2)) + mlp_t2(u2)
        v2 = u2 * exp(mlp_s1(v1)) + mlp_t1(v1)
        u1, u2 = v1, v2
    out = [u1 | u2 | t]
Each mlp is 256 -> 32 (tanh) -> 256.

v3 strategy (pure data parallel over batch, 131072 -> 8 cores x 16384):
  * Host-side batch transpose: each core gets x_t [512, bc] feature-major and
    returns out [512, bc]; the t column never touches the device.
  * float32r matmuls (4x PE rate when warm).
  * TWO batch chains (A, B) are processed as one "pair": their L1 matmuls
    col-tile the PE (M=64 each at col offsets 0/64) into ONE psum bank, so a
    single tanh instruction activates both chains' hidden layers.  th layout
    is [sA tA sB tB] x 32 rows, which also gives the eight K=32 second-layer
    matmuls 4-way row-group concurrency.
  * Measured per-instruction costs (ACT ~250ns + stream, DVE ~240ns + stream,
    GPSIMD ~2.9 cyc/elem) make instruction count the scarce resource, so all
    elementwise ops run pair-wide ([128, 2, 512] or bigger):
      - exp fc0: one ACT instr with b2s bias;
      - exp fc1: one ACT instr, no bias -- exp(b2s) is folded into the fc1
        multiply, which runs as a DVE scalar_tensor_tensor (u*eb2s)*ee;
      - fc0 multiply: GPSIMD tensor_mul (SBUF-only operands);
      - readout v = tmp + t_psum + b2t: DVE scalar_tensor_tensor per
        [128,512] slice; one slice per half-step is instead routed through a
        PE identity-matmul accumulate (tmp += into the t psum bank) and an
        ACT Identity(psum + b2t) copy, to balance DVE vs ACT load.
  * PSUM: tag "h" [128,512] bufs=2 (2 banks) + tag "st" [128,2ch,512] bufs=3
    (6 banks) = 8 banks; the s tiles are freed by exp before the t tiles are
    allocated, so a 3-deep ring covers s-fc0, s-fc1, t-fc0, t-fc1 per step.
"""

from contextlib import ExitStack

import numpy as np

import concourse.bass as bass
import concourse.tile as tile
from concourse import bacc, mybir
from concourse.bass_utils import run_bass_kernel_spmd

F32 = mybir.dt.float32
F32R = mybir.dt.float32r

USE_F32R = True      # all matmuls in float32r
ACT_READOUT = True   # route one readout slice per half-step via PE+ACT

B_TOTAL = 131072
D = 512
S = 256
H = 32
L = 3
NCORES = 8
BT = 512  # batch columns per chain-tile (= one PSUM bank of fp32)

MMDT = F32R if USE_F32R else F32


def _f32(ap):
    """View a float32r AP as plain float32 for non-matmul consumers."""
    return ap.bitcast(F32) if USE_F32R else ap


def _pack_weights(W1, b1, W2, b2):
    """Host-side repack into PE-friendly layouts for the paired design.

    q=0 updates u1 from u2 (s-idx 1, t-idx 3); q=1 updates u2 from v1
    (s-idx 0, t-idx 2).  First layer is unduplicated (M=64: [s|t] cols);
    second-layer rows are packed [s t s t] so chains A and B own row
    groups (0,32) and (64,96).
    """
    W1 = np.asarray(W1, np.float32)
    b1 = np.asarray(b1, np.float32)
    W2 = np.asarray(W2, np.float32)
    b2 = np.asarray(b2, np.float32)
    w1p = np.empty((L, 2, 2, 128, 64), np.float32)
    b1p = np.empty((L, 2, 128), np.float32)
    w2p = np.empty((L, 2, 128, 256), np.float32)
    b2sp = np.empty((L, 2, 128), np.float32)    # exp bias for fc0
    eb2sp = np.empty((L, 2, 128), np.float32)   # exp(b2s) scale for fc1
    b2tp = np.empty((L, 2, 128, 2), np.float32)
    for j in range(L):
        for q in range(2):
            s_idx, t_idx = (1, 3) if q == 0 else (0, 2)
            for c in range(2):
                blk = slice(c * 128, (c + 1) * 128)
                w1p[j, q, c, :, 0:32] = W1[j, s_idx, blk, :]
                w1p[j, q, c, :, 32:64] = W1[j, t_idx, blk, :]
            for g in range(2):
                b1p[j, q, 64 * g : 64 * g + 32] = b1[j, s_idx]
                b1p[j, q, 64 * g + 32 : 64 * g + 64] = b1[j, t_idx]
                w2p[j, q, 64 * g : 64 * g + 32, :] = W2[j, s_idx]
                w2p[j, q, 64 * g + 32 : 64 * g + 64, :] = W2[j, t_idx]
            b2sp[j, q] = b2[j, s_idx, 0:128]
            eb2sp[j, q] = np.exp(b2[j, s_idx, 128:256])
            b2tp[j, q, :, 0] = b2[j, t_idx, 0:128]
            b2tp[j, q, :, 1] = b2[j, t_idx, 128:256]
    ident = np.eye(128, dtype=np.float32)
    return dict(
        w1p=w1p, b1p=b1p, w2p=w2p, b2sp=b2sp, eb2sp=eb2sp, b2tp=b2tp,
        ident=ident,
    )


def build_nc(bc):
    """Per-core Bass program; x_t [512, bc] feature-major in, out [512, bc]."""
    assert bc % (2 * BT) == 0
    npair = bc // (2 * BT)
    nc = bacc.Bacc(None, target_bir_lowering=False)
    x_d = nc.declare_dram_parameter("x_t", [D, bc], MMDT, isOutput=False)
    w1_d = nc.declare_dram_parameter("w1p", [L, 2, 2, 128, 64], F32, isOutput=False)
    b1_d = nc.declare_dram_parameter("b1p", [L, 2, 128], F32, isOutput=False)
    w2_d = nc.declare_dram_parameter("w2p", [L, 2, 128, 256], MMDT, isOutput=False)
    b2s_d = nc.declare_dram_parameter("b2sp", [L, 2, 128], F32, isOutput=False)
    eb2s_d = nc.declare_dram_parameter("eb2sp", [L, 2, 128], F32, isOutput=False)
    b2t_d = nc.declare_dram_parameter("b2tp", [L, 2, 128, 2], F32, isOutput=False)
    id_d = nc.declare_dram_parameter("ident", [128, 128], MMDT, isOutput=False)
    out_d = nc.declare_dram_parameter("out", [D, bc], MMDT, isOutput=True)

    TANH = mybir.ActivationFunctionType.Tanh
    EXP = mybir.ActivationFunctionType.Exp
    IDENT = mybir.ActivationFunctionType.Identity
    ADD = mybir.AluOpType.add
    MULT = mybir.AluOpType.mult

    with tile.TileContext(nc) as tc, ExitStack() as ctx:
        singles = ctx.enter_context(tc.tile_pool(name="singles", bufs=1))
        p_state = ctx.enter_context(tc.tile_pool(name="state", bufs=4))
        p_th = ctx.enter_context(tc.tile_pool(name="th", bufs=3))
        p_e = ctx.enter_context(tc.tile_pool(name="e", bufs=3))
        p_tmp = ctx.enter_context(tc.tile_pool(name="tmp", bufs=3))
        ps_h = ctx.enter_context(
            tc.tile_pool(name="ps_h", bufs=2, space=bass.MemorySpace.PSUM)
        )
        ps_st = ctx.enter_context(
            tc.tile_pool(name="ps_st", bufs=3, space=bass.MemorySpace.PSUM)
        )

        # --- weights (persist in SBUF) -----------------------------------
        # first layer runs in plain fp32: float32r cannot col-tile (walrus
        # s3d3_mm_valid_dst_partition), and exactness here is free PE slack
        w1s = singles.tile([128, L, 2, 2, 64], F32)
        nc.gpsimd.dma_start(
            out=w1s[:], in_=w1_d[:].rearrange("j q c p m -> p j q c m")
        )
        b1s = singles.tile([128, L, 2], F32)
        nc.gpsimd.dma_start(out=b1s[:], in_=b1_d[:].rearrange("j q p -> p j q"))
        w2s = singles.tile([128, L, 2, 256], MMDT)
        nc.gpsimd.dma_start(
            out=w2s[:], in_=w2_d[:].rearrange("j q p m -> p j q m")
        )
        b2ss = singles.tile([128, L, 2], F32)
        nc.gpsimd.dma_start(out=b2ss[:], in_=b2s_d[:].rearrange("j q p -> p j q"))
        eb2ss = singles.tile([128, L, 2], F32)
        nc.gpsimd.dma_start(out=eb2ss[:], in_=eb2s_d[:].rearrange("j q p -> p j q"))
        b2ts = singles.tile([128, L, 2, 2], F32)
        nc.gpsimd.dma_start(out=b2ts[:], in_=b2t_d[:].rearrange("j q p c -> p j q c"))
        ids = singles.tile([128, 128], MMDT)
        nc.gpsimd.dma_start(out=ids[:], in_=id_d[:])

        for pair in range(npair):
            b0 = pair * 2 * BT
            # paired state tiles: [128, chain(2), fc(2), BT]
            u = []
            for h in range(2):
                ut = p_state.tile([128, 2, 2, BT], MMDT, tag=f"st{h}")
                for ch in range(2):
                    bch = b0 + ch * BT
                    nc.sync.dma_start(
                        out=ut[:, ch, :, :],
                        in_=x_d[h * S : (h + 1) * S, bch : bch + BT].rearrange(
                            "(c p) b -> p c b", p=128
                        ),
                    )
                u.append(ut)

            for j in range(L):
                for q in range(2):
                    hin = u[1 - q]
                    tgt = u[q]
                    # L1: col-tiled M=64 per chain into one shared psum bank
                    ph = ps_h.tile([128, BT], F32, tag="h")
                    for ch in range(2):
                        for c in range(2):
                            nc.tensor.matmul(
                                ph[64 * ch : 64 * ch + 64, :],
                                w1s[:, j, q, c, :],
                                _f32(hin[:, ch, c, :]),
                                start=(c == 0),
                                stop=(c == 1),
                                tile_position=(0, 64 * ch),
                            )
                    th = p_th.tile([128, BT], MMDT, tag="th")
                    nc.scalar.activation(
                        th[:], ph[:], TANH, bias=b1s[:, j, q : q + 1]
                    )
                    # L2: s then t, 4-way row-group concurrency per wave
                    pss = {}
                    for fc in range(2):
                        ps = ps_st.tile([128, 2, BT], F32, tag="st")
                        for ch in range(2):
                            r = 64 * ch
                            nc.tensor.matmul(
                                ps[:, ch, :],
                                w2s[r : r + 32, j, q, fc * 128 : (fc + 1) * 128],
                                th[r : r + 32, :],
                                tile_position=(r, 0),
                            )
                        pss[fc] = ps
                    ee = p_e.tile([128, 2, 2, BT], F32, tag="e")
                    nc.scalar.activation(
                        ee[:, :, 0, :], pss[0][:], EXP, bias=b2ss[:, j, q : q + 1]
                    )
                    nc.scalar.activation(ee[:, :, 1, :], pss[1][:], EXP)
                    pst = {}
                    for fc in range(2):
                        ps = ps_st.tile([128, 2, BT], F32, tag="st")
                        for ch in range(2):
                            r = 64 * ch + 32
                            acc = ACT_READOUT and ch == 1 and fc == 1
                            nc.tensor.matmul(
                                ps[:, ch, :],
                                w2s[r : r + 32, j, q, fc * 128 : (fc + 1) * 128],
                                th[r : r + 32, :],
                                tile_position=(r, 0),
                                start=True,
                                stop=not acc,
                            )
                        pst[fc] = ps
                    # fc0 multiply on GPSIMD (plain; b2s went through exp bias)
                    tmp0 = p_tmp.tile([128, 2, BT], F32, tag="tmp0")
                    nc.gpsimd.tensor_mul(
                        out=tmp0[:],
                        in0=_f32(tgt[:, :, 0, :]),
                        in1=ee[:, :, 0, :],
                    )
                    # fc1 multiply on DVE with exp(b2s) folded in
                    tmp1 = p_tmp.tile([128, 2, BT], MMDT, tag="tmp1")
                    nc.vector.scalar_tensor_tensor(
                        out=tmp1[:],
                        in0=_f32(tgt[:, :, 1, :]),
                        scalar=eb2ss[:, j, q : q + 1],
                        in1=ee[:, :, 1, :],
                        op0=MULT,
                        op1=MULT,
                    )
                    tmp = {0: tmp0, 1: tmp1}
                    v = p_state.tile([128, 2, 2, BT], MMDT, tag=f"st{q}")
                    for ch in range(2):
                        for fc in range(2):
                            if ACT_READOUT and ch == 1 and fc == 1:
                                # PE accumulates tmp into the t psum bank,
                                # ACT drains it (+b2t) to SBUF
                                nc.tensor.matmul(
                                    pst[fc][:, ch, :],
                                    ids[:],
                                    tmp1[:, ch, :],
                                    start=False,
                                    stop=True,
                                )
                                nc.scalar.activation(
                                    v[:, ch, fc, :],
                                    pst[fc][:, ch, :],
                                    IDENT,
                                    bias=b2ts[:, j, q, fc : fc + 1],
                                )
                            else:
                                nc.vector.scalar_tensor_tensor(
                                    out=v[:, ch, fc, :],
                                    in0=_f32(tmp[fc][:, ch, :]),
                                    scalar=b2ts[:, j, q, fc : fc + 1],
                                    in1=pst[fc][:, ch, :],
                                    op0=ADD,
                                    op1=ADD,
                                )
                    u[q] = v

            for h in range(2):
                for ch in range(2):
                    bch = b0 + ch * BT
                    nc.sync.dma_start(
                        out=out_d[h * S : (h + 1) * S, bch : bch + BT].rearrange(
                            "(c p) b -> p c b", p=128
                        ),
                        in_=u[h][:, ch, :, :],
                    )
    nc.compile()
    return nc


_NC_CACHE = {}
TRACE = False
LAST_EXEC_NS = None
LAST_RES = None


def _get_nc(bc):
    if bc not in _NC_CACHE:
        _NC_CACHE[bc] = build_nc(bc)
    return _NC_CACHE[bc]


def kernel(x, W1, b1, W2, b2):
    global LAST_EXEC_NS
    x = np.asarray(x, np.float32)
    b = x.shape[0]
    assert b % NCORES == 0
    bc = b // NCORES
    packed = _pack_weights(W1, b1, W2, b2)
    nc = _get_nc(bc)
    in_maps = [
        {
            "x_t": np.ascontiguousarray(x[i * bc : (i + 1) * bc, :D].T),
            **packed,
        }
        for i in range(NCORES)
    ]
    res = run_bass_kernel_spmd(nc, in_maps, list(range(NCORES)), trace=TRACE)
    if getattr(res, "exec_time_ns", None):
        LAST_EXEC_NS = res.exec_time_ns
    if TRACE:
        globals()["LAST_RES"] = res
    out = np.empty((b, D + 1), np.float32)
    for i in range(NCORES):
        out[i * bc : (i + 1) * bc, :D] = res.results[i]["out"].T
    out[:, D] = x[:, D]
    return out


# revision 18
# speedup vs baseline: 1.1704x; 1.1704x over previous
"""Trainium2 Bass kernel for the 3-block invertible coupling flow (RealNVP-style).

Computation (per sample row of x = [u1(256) | u2(256) | t(1)]):
    for j in 3 blocks:
        v1 = u1 * exp(mlp_s2(u2)) + mlp_t2(u2)
        v2 = u2 * exp(mlp_s1(v1)) + mlp_t1(v1)
        u1, u2 = v1, v2
    out = [u1 | u2 | t]
Each mlp is 256 -> 32 (tanh) -> 256.

v4 strategy (pure data parallel over batch, 131072 -> 8 cores x 16384):
  * Host-side batch transpose: each core gets x_t [512, bc] feature-major and
    returns out [512, bc]; the t column never touches the device.
  * All matmuls float32r (1 cycle/row when the PE is warm).  Hidden units are
    duplicated [s,s,t,t] in the first layer so each chain's K=32 second-layer
    matmuls come from its own 32-row groups: with chains A/B assigned groups
    (0,64) and (32,96), all four run concurrently via tile_position.
  * TWO batch chains form a "pair" sharing one instruction per elementwise
    stage (instruction overhead, not streaming, dominates ACT/DVE):
      - L1 writes ph_pair [128, 2ch, 512] (chain = bank); ONE tanh instr;
      - L2 s/t outputs land in [128, 2ch, 512] psum tiles per fc;
      - exp fc0 (bias=b2s) and fc1 (bias-free; exp(b2s) folded into the fc1
        multiply) are ONE ACT instr each;
      - fc0 multiply: GPSIMD tensor_mul [128,2,512] (SBUF only);
      - fc1 multiply: DVE scalar_tensor_tensor (u*eb2s)*ee;
      - readout per fc: ONE DVE scalar_tensor_tensor (tmp+b2t)+t_psum over
        [128, 2ch, 512] (b2t is per-feature, so identical across chains).
  * PSUM: tag "h" [128,2,512] bufs=1 (2 banks) + tag "st" bufs=3 (6) = 8.
"""

from contextlib import ExitStack

import numpy as np

import concourse.bass as bass
import concourse.tile as tile
from concourse import bacc, mybir
from concourse.bass_utils import run_bass_kernel_spmd

F32 = mybir.dt.float32
F32R = mybir.dt.float32r

USE_F32R = True

B_TOTAL = 131072
D = 512
S = 256
H = 32
L = 3
NCORES = 8
BT = 512  # batch columns per chain-tile (= one PSUM bank of fp32)

MMDT = F32R if USE_F32R else F32


def _f32(ap):
    """View a float32r AP as plain float32 for non-matmul consumers."""
    return ap.bitcast(F32) if USE_F32R else ap


def _pack_weights(W1, b1, W2, b2):
    """Host-side repack of the MLP weights (hidden duplicated [s,s,t,t]).

    q=0 updates u1 from u2 (s-idx 1, t-idx 3); q=1 updates u2 from v1
    (s-idx 0, t-idx 2).
    """
    W1 = np.asarray(W1, np.float32)
    b1 = np.asarray(b1, np.float32)
    W2 = np.asarray(W2, np.float32)
    b2 = np.asarray(b2, np.float32)
    w1p = np.empty((L, 2, 2, 128, 128), np.float32)
    b1p = np.empty((L, 2, 128), np.float32)
    w2p = np.empty((L, 2, 128, 256), np.float32)
    b2sp = np.empty((L, 2, 128), np.float32)    # exp bias for fc0
    eb2sp = np.empty((L, 2, 128), np.float32)   # exp(b2s) scale for fc1
    b2tp = np.empty((L, 2, 128, 2), np.float32)
    for j in range(L):
        for q in range(2):
            s_idx, t_idx = (1, 3) if q == 0 else (0, 2)
            for c in range(2):
                blk = slice(c * 128, (c + 1) * 128)
                w1p[j, q, c, :, 0:32] = W1[j, s_idx, blk, :]
                w1p[j, q, c, :, 32:64] = W1[j, s_idx, blk, :]
                w1p[j, q, c, :, 64:96] = W1[j, t_idx, blk, :]
                w1p[j, q, c, :, 96:128] = W1[j, t_idx, blk, :]
            b1p[j, q, 0:32] = b1[j, s_idx]
            b1p[j, q, 32:64] = b1[j, s_idx]
            b1p[j, q, 64:96] = b1[j, t_idx]
            b1p[j, q, 96:128] = b1[j, t_idx]
            w2p[j, q, 0:32, :] = W2[j, s_idx]
            w2p[j, q, 32:64, :] = W2[j, s_idx]
            w2p[j, q, 64:96, :] = W2[j, t_idx]
            w2p[j, q, 96:128, :] = W2[j, t_idx]
            b2sp[j, q] = b2[j, s_idx, 0:128]
            eb2sp[j, q] = np.exp(b2[j, s_idx, 128:256])
            b2tp[j, q, :, 0] = b2[j, t_idx, 0:128]
            b2tp[j, q, :, 1] = b2[j, t_idx, 128:256]
    return dict(w1p=w1p, b1p=b1p, w2p=w2p, b2sp=b2sp, eb2sp=eb2sp, b2tp=b2tp)


def build_nc(bc):
    """Per-core Bass program; x_t [512, bc] feature-major in, out [512, bc]."""
    assert bc % (2 * BT) == 0
    npair = bc // (2 * BT)
    nc = bacc.Bacc(None, target_bir_lowering=False)
    x_d = nc.declare_dram_parameter("x_t", [D, bc], MMDT, isOutput=False)
    w1_d = nc.declare_dram_parameter("w1p", [L, 2, 2, 128, 128], MMDT, isOutput=False)
    b1_d = nc.declare_dram_parameter("b1p", [L, 2, 128], F32, isOutput=False)
    w2_d = nc.declare_dram_parameter("w2p", [L, 2, 128, 256], MMDT, isOutput=False)
    b2s_d = nc.declare_dram_parameter("b2sp", [L, 2, 128], F32, isOutput=False)
    eb2s_d = nc.declare_dram_parameter("eb2sp", [L, 2, 128], F32, isOutput=False)
    b2t_d = nc.declare_dram_parameter("b2tp", [L, 2, 128, 2], F32, isOutput=False)
    out_d = nc.declare_dram_parameter("out", [D, bc], MMDT, isOutput=True)

    TANH = mybir.ActivationFunctionType.Tanh
    EXP = mybir.ActivationFunctionType.Exp
    ADD = mybir.AluOpType.add
    MULT = mybir.AluOpType.mult

    with tile.TileContext(nc) as tc, ExitStack() as ctx:
        singles = ctx.enter_context(tc.tile_pool(name="singles", bufs=1))
        p_state = ctx.enter_context(tc.tile_pool(name="state", bufs=4))
        p_th = ctx.enter_context(tc.tile_pool(name="th", bufs=3))
        p_e = ctx.enter_context(tc.tile_pool(name="e", bufs=3))
        p_tmp = ctx.enter_context(tc.tile_pool(name="tmp", bufs=3))
        ps_h = ctx.enter_context(
            tc.tile_pool(name="ps_h", bufs=1, space=bass.MemorySpace.PSUM)
        )
        ps_st = ctx.enter_context(
            tc.tile_pool(name="ps_st", bufs=3, space=bass.MemorySpace.PSUM)
        )

        # --- weights (persist in SBUF) -----------------------------------
        w1s = singles.tile([128, L, 2, 2, 128], MMDT)
        nc.gpsimd.dma_start(
            out=w1s[:], in_=w1_d[:].rearrange("j q c p m -> p j q c m")
        )
        b1s = singles.tile([128, L, 2], F32)
        nc.gpsimd.dma_start(out=b1s[:], in_=b1_d[:].rearrange("j q p -> p j q"))
        w2s = singles.tile([128, L, 2, 256], MMDT)
        nc.gpsimd.dma_start(
            out=w2s[:], in_=w2_d[:].rearrange("j q p m -> p j q m")
        )
        b2ss = singles.tile([128, L, 2], F32)
        nc.gpsimd.dma_start(out=b2ss[:], in_=b2s_d[:].rearrange("j q p -> p j q"))
        eb2ss = singles.tile([128, L, 2], F32)
        nc.gpsimd.dma_start(out=eb2ss[:], in_=eb2s_d[:].rearrange("j q p -> p j q"))
        b2ts = singles.tile([128, L, 2, 2], F32)
        nc.gpsimd.dma_start(out=b2ts[:], in_=b2t_d[:].rearrange("j q p c -> p j q c"))

        for pair in range(npair):
            b0 = pair * 2 * BT
            # paired state tiles: [128, chain(2), fc(2), BT]
            u = []
            for h in range(2):
                ut = p_state.tile([128, 2, 2, BT], MMDT, tag=f"st{h}")
                for ch in range(2):
                    bch = b0 + ch * BT
                    nc.sync.dma_start(
                        out=ut[:, ch, :, :],
                        in_=x_d[h * S : (h + 1) * S, bch : bch + BT].rearrange(
                            "(c p) b -> p c b", p=128
                        ),
                    )
                u.append(ut)

            for j in range(L):
                for q in range(2):
                    hin = u[1 - q]
                    tgt = u[q]
                    # L1 per chain (M=128, duplicated hidden), shared psum tile
                    ph = ps_h.tile([128, 2, BT], F32, tag="h")
                    for ch in range(2):
                        for c in range(2):
                            nc.tensor.matmul(
                                ph[:, ch, :],
                                w1s[:, j, q, c, :],
                                hin[:, ch, c, :],
                                start=(c == 0),
                                stop=(c == 1),
                            )
                    th = p_th.tile([128, 2, BT], MMDT, tag="th")
                    nc.scalar.activation(
                        th[:], ph[:], TANH, bias=b1s[:, j, q : q + 1]
                    )
                    # L2: chains use disjoint row groups via the duplication:
                    # A reads its rows (0,64), B its rows (32,96)
                    pss = {}
                    pst = {}
                    for fc in range(2):
                        pss[fc] = ps_st.tile(
                            [128, 2, BT], F32, tag="st", name=f"pss{fc}"
                        )
                        for ch in range(2):
                            r = 32 * ch
                            nc.tensor.matmul(
                                pss[fc][:, ch, :],
                                w2s[r : r + 32, j, q, fc * 128 : (fc + 1) * 128],
                                th[r : r + 32, ch, :],
                                tile_position=(r, 0),
                            )
                    for fc in range(2):
                        pst[fc] = ps_st.tile(
                            [128, 2, BT], F32, tag="st", name=f"pst{fc}"
                        )
                        for ch in range(2):
                            r = 64 + 32 * ch
                            nc.tensor.matmul(
                                pst[fc][:, ch, :],
                                w2s[r : r + 32, j, q, fc * 128 : (fc + 1) * 128],
                                th[r : r + 32, ch, :],
                                tile_position=(r, 0),
                            )
                    ee = p_e.tile([128, 2, 2, BT], F32, tag="e")
                    nc.scalar.activation(
                        ee[:, :, 0, :], pss[0][:], EXP, bias=b2ss[:, j, q : q + 1]
                    )
                    nc.scalar.activation(ee[:, :, 1, :], pss[1][:], EXP)
                    # fc0 multiply on GPSIMD (plain; b2s went through exp bias)
                    tmp0 = p_tmp.tile([128, 2, BT], F32, tag="tmp0")
                    nc.gpsimd.tensor_mul(
                        out=tmp0[:],
                        in0=_f32(tgt[:, :, 0, :]),
                        in1=ee[:, :, 0, :],
                    )
                    # fc1 multiply on DVE with exp(b2s) folded in
                    tmp1 = p_tmp.tile([128, 2, BT], MMDT, tag="tmp1")
                    nc.vector.scalar_tensor_tensor(
                        out=tmp1[:],
                        in0=_f32(tgt[:, :, 1, :]),
                        scalar=eb2ss[:, j, q : q + 1],
                        in1=ee[:, :, 1, :],
                        op0=MULT,
                        op1=MULT,
                    )
                    tmp = {0: tmp0, 1: tmp1}
                    v = p_state.tile([128, 2, 2, BT], MMDT, tag=f"st{q}")
                    for fc in range(2):
                        nc.vector.scalar_tensor_tensor(
                            out=v[:, :, fc, :],
                            in0=_f32(tmp[fc][:]),
                            scalar=b2ts[:, j, q, fc : fc + 1],
                            in1=pst[fc][:],
                            op0=ADD,
                            op1=ADD,
                        )
                    u[q] = v

            for h in range(2):
                for ch in range(2):
                    bch = b0 + ch * BT
                    nc.sync.dma_start(
                        out=out_d[h * S : (h + 1) * S, bch : bch + BT].rearrange(
                            "(c p) b -> p c b", p=128
                        ),
                        in_=u[h][:, ch, :, :],
                    )
    nc.compile()
    return nc


_NC_CACHE = {}
TRACE = False
LAST_EXEC_NS = None
LAST_RES = None


def _get_nc(bc):
    if bc not in _NC_CACHE:
        _NC_CACHE[bc] = build_nc(bc)
    return _NC_CACHE[bc]


def kernel(x, W1, b1, W2, b2):
    global LAST_EXEC_NS
    x = np.asarray(x, np.float32)
    b = x.shape[0]
    assert b % NCORES == 0
    bc = b // NCORES
    packed = _pack_weights(W1, b1, W2, b2)
    nc = _get_nc(bc)
    in_maps = [
        {
            "x_t": np.ascontiguousarray(x[i * bc : (i + 1) * bc, :D].T),
            **packed,
        }
        for i in range(NCORES)
    ]
    res = run_bass_kernel_spmd(nc, in_maps, list(range(NCORES)), trace=TRACE)
    if getattr(res, "exec_time_ns", None):
        LAST_EXEC_NS = res.exec_time_ns
    if TRACE:
        globals()["LAST_RES"] = res
    out = np.empty((b, D + 1), np.float32)
    for i in range(NCORES):
        out[i * bc : (i + 1) * bc, :D] = res.results[i]["out"].T
    out[:, D] = x[:, D]
    return out


# revision 25
# speedup vs baseline: 1.8751x; 1.6020x over previous
"""Trainium2 Bass kernel for the 3-block invertible coupling flow (RealNVP-style).

Computation (per sample row of x = [u1(256) | u2(256) | t(1)]):
    for j in 3 blocks:
        v1 = u1 * exp(mlp_s2(u2)) + mlp_t2(u2)
        v2 = u2 * exp(mlp_s1(v1)) + mlp_t1(v1)
        u1, u2 = v1, v2
    out = [u1 | u2 | t]
Each mlp is 256 -> 32 (tanh) -> 256.

v4 strategy (pure data parallel over batch, 131072 -> 8 cores x 16384):
  * Host-side batch transpose: each core gets x_t [512, bc] feature-major and
    returns out [512, bc]; the t column never touches the device.
  * All matmuls float32r (1 cycle/row when the PE is warm).  Hidden units are
    duplicated [s,s,t,t] in the first layer so each chain's K=32 second-layer
    matmuls come from its own 32-row groups: with chains A/B assigned groups
    (0,64) and (32,96), all four run concurrently via tile_position.
  * TWO batch chains form a "pair" sharing one instruction per elementwise
    stage (instruction overhead, not streaming, dominates ACT/DVE):
      - L1 writes ph_pair [128, 2ch, 512] (chain = bank); ONE tanh instr;
      - L2 s/t outputs land in [128, 2ch, 512] psum tiles per fc;
      - exp fc0 (bias=b2s) and fc1 (bias-free; exp(b2s) folded into the fc1
        multiply) are ONE ACT instr each;
      - fc0 multiply: GPSIMD tensor_mul [128,2,512] (SBUF only);
      - fc1 multiply: DVE scalar_tensor_tensor (u*eb2s)*ee;
      - readout per fc: ONE DVE scalar_tensor_tensor (tmp+b2t)+t_psum over
        [128, 2ch, 512] (b2t is per-feature, so identical across chains).
  * PSUM: tag "h" [128,2,512] bufs=1 (2 banks) + tag "st" bufs=3 (6) = 8.
"""

from contextlib import ExitStack

import numpy as np

import concourse.bass as bass
import concourse.tile as tile
from concourse import bacc, mybir
from concourse.bass_utils import run_bass_kernel_spmd

F32 = mybir.dt.float32
F32R = mybir.dt.float32r

USE_F32R = True

B_TOTAL = 131072
D = 512
S = 256
H = 32
L = 3
NCORES = 8
BT = 512  # batch columns per chain-tile (= one PSUM bank of fp32)

MMDT = F32R if USE_F32R else F32


def _f32(ap):
    """View a float32r AP as plain float32 for non-matmul consumers."""
    return ap.bitcast(F32) if USE_F32R else ap


def _pack_weights(W1, b1, W2, b2):
    """Host-side repack of the MLP weights (hidden duplicated [s,s,t,t]).

    q=0 updates u1 from u2 (s-idx 1, t-idx 3); q=1 updates u2 from v1
    (s-idx 0, t-idx 2).
    """
    W1 = np.asarray(W1, np.float32)
    b1 = np.asarray(b1, np.float32)
    W2 = np.asarray(W2, np.float32)
    b2 = np.asarray(b2, np.float32)
    w1p = np.empty((L, 2, 2, 128, 128), np.float32)
    b1p = np.empty((L, 2, 128), np.float32)
    w2p = np.empty((L, 2, 128, 256), np.float32)
    b2sp = np.empty((L, 2, 128), np.float32)    # exp bias for fc0
    eb2sp = np.empty((L, 2, 128), np.float32)   # exp(b2s) scale for fc1
    b2tp = np.empty((L, 2, 128, 2), np.float32)
    for j in range(L):
        for q in range(2):
            s_idx, t_idx = (1, 3) if q == 0 else (0, 2)
            for c in range(2):
                blk = slice(c * 128, (c + 1) * 128)
                w1p[j, q, c, :, 0:32] = W1[j, s_idx, blk, :]
                w1p[j, q, c, :, 32:64] = W1[j, s_idx, blk, :]
                w1p[j, q, c, :, 64:96] = W1[j, t_idx, blk, :]
                w1p[j, q, c, :, 96:128] = W1[j, t_idx, blk, :]
            b1p[j, q, 0:32] = b1[j, s_idx]
            b1p[j, q, 32:64] = b1[j, s_idx]
            b1p[j, q, 64:96] = b1[j, t_idx]
            b1p[j, q, 96:128] = b1[j, t_idx]
            w2p[j, q, 0:32, :] = W2[j, s_idx]
            w2p[j, q, 32:64, :] = W2[j, s_idx]
            w2p[j, q, 64:96, :] = W2[j, t_idx]
            w2p[j, q, 96:128, :] = W2[j, t_idx]
            b2sp[j, q] = b2[j, s_idx, 0:128]
            eb2sp[j, q] = np.exp(b2[j, s_idx, 128:256])
            b2tp[j, q, :, 0] = b2[j, t_idx, 0:128]
            b2tp[j, q, :, 1] = b2[j, t_idx, 128:256]
    return dict(w1p=w1p, b1p=b1p, w2p=w2p, b2sp=b2sp, eb2sp=eb2sp, b2tp=b2tp)


def build_nc(bc):
    """Per-core Bass program; x_t [512, bc] feature-major in, out [512, bc]."""
    assert bc % (2 * BT) == 0
    npair = bc // (2 * BT)
    nc = bacc.Bacc(None, target_bir_lowering=False)
    x_d = nc.declare_dram_parameter("x_t", [D, bc], MMDT, isOutput=False)
    w1_d = nc.declare_dram_parameter("w1p", [L, 2, 2, 128, 128], MMDT, isOutput=False)
    b1_d = nc.declare_dram_parameter("b1p", [L, 2, 128], F32, isOutput=False)
    w2_d = nc.declare_dram_parameter("w2p", [L, 2, 128, 256], MMDT, isOutput=False)
    b2s_d = nc.declare_dram_parameter("b2sp", [L, 2, 128], F32, isOutput=False)
    eb2s_d = nc.declare_dram_parameter("eb2sp", [L, 2, 128], F32, isOutput=False)
    b2t_d = nc.declare_dram_parameter("b2tp", [L, 2, 128, 2], F32, isOutput=False)
    out_d = nc.declare_dram_parameter("out", [D, bc], MMDT, isOutput=True)

    TANH = mybir.ActivationFunctionType.Tanh
    EXP = mybir.ActivationFunctionType.Exp
    ADD = mybir.AluOpType.add
    MULT = mybir.AluOpType.mult

    with tile.TileContext(nc) as tc, ExitStack() as ctx:
        singles = ctx.enter_context(tc.tile_pool(name="singles", bufs=1))
        p_state = ctx.enter_context(tc.tile_pool(name="state", bufs=4))
        p_th = ctx.enter_context(tc.tile_pool(name="th", bufs=3))
        p_e = ctx.enter_context(tc.tile_pool(name="e", bufs=3))
        p_tmp = ctx.enter_context(tc.tile_pool(name="tmp", bufs=3))
        ps_h = ctx.enter_context(
            tc.tile_pool(name="ps_h", bufs=1, space=bass.MemorySpace.PSUM)
        )
        ps_st = ctx.enter_context(
            tc.tile_pool(name="ps_st", bufs=3, space=bass.MemorySpace.PSUM)
        )

        # --- weights (persist in SBUF) -----------------------------------
        w1s = singles.tile([128, L, 2, 2, 128], MMDT)
        nc.gpsimd.dma_start(
            out=w1s[:], in_=w1_d[:].rearrange("j q c p m -> p j q c m")
        )
        b1s = singles.tile([128, L, 2], F32)
        nc.gpsimd.dma_start(out=b1s[:], in_=b1_d[:].rearrange("j q p -> p j q"))
        w2s = singles.tile([128, L, 2, 256], MMDT)
        nc.gpsimd.dma_start(
            out=w2s[:], in_=w2_d[:].rearrange("j q p m -> p j q m")
        )
        b2ss = singles.tile([128, L, 2], F32)
        nc.gpsimd.dma_start(out=b2ss[:], in_=b2s_d[:].rearrange("j q p -> p j q"))
        eb2ss = singles.tile([128, L, 2], F32)
        nc.gpsimd.dma_start(out=eb2ss[:], in_=eb2s_d[:].rearrange("j q p -> p j q"))
        b2ts = singles.tile([128, L, 2, 2], F32)
        nc.gpsimd.dma_start(out=b2ts[:], in_=b2t_d[:].rearrange("j q p c -> p j q c"))

        assert npair % 2 == 0
        for sup in range(npair // 2):
            prs = (2 * sup, 2 * sup + 1)
            # paired state tiles: [128, chain(2), fc(2), BT]; two pairs (four
            # chains) are interleaved per half-step so every engine queue
            # always holds the other pair's independent, ready instructions
            us = {}
            for pr in prs:
                b0 = pr * 2 * BT
                u = []
                for h in range(2):
                    ut = p_state.tile(
                        [128, 2, 2, BT], MMDT, tag=f"st{h}{pr % 2}", name=f"ut{h}{pr}"
                    )
                    for ch in range(2):
                        bch = b0 + ch * BT
                        nc.sync.dma_start(
                            out=ut[:, ch, :, :],
                            in_=x_d[h * S : (h + 1) * S, bch : bch + BT].rearrange(
                                "(c p) b -> p c b", p=128
                            ),
                        )
                    u.append(ut)
                us[pr] = u

            for j in range(L):
                for q in range(2):
                  for pr in prs:
                    u = us[pr]
                    hin = u[1 - q]
                    tgt = u[q]
                    # L1 per chain (M=128, duplicated hidden), shared psum tile
                    ph = ps_h.tile([128, 2, BT], F32, tag="h")
                    for ch in range(2):
                        for c in range(2):
                            nc.tensor.matmul(
                                ph[:, ch, :],
                                w1s[:, j, q, c, :],
                                hin[:, ch, c, :],
                                start=(c == 0),
                                stop=(c == 1),
                            )
                    th = p_th.tile([128, 2, BT], MMDT, tag="th")
                    nc.scalar.activation(
                        th[:], ph[:], TANH, bias=b1s[:, j, q : q + 1]
                    )
                    # L2: chains use disjoint row groups via the duplication:
                    # A reads its rows (0,64), B its rows (32,96)
                    pss = {}
                    pst = {}
                    for fc in range(2):
                        pss[fc] = ps_st.tile(
                            [128, 2, BT], F32, tag="st", name=f"pss{fc}"
                        )
                        for ch in range(2):
                            r = 32 * ch
                            nc.tensor.matmul(
                                pss[fc][:, ch, :],
                                w2s[r : r + 32, j, q, fc * 128 : (fc + 1) * 128],
                                th[r : r + 32, ch, :],
                                tile_position=(r, 0),
                            )
                    ee = p_e.tile([128, 2, 2, BT], F32, tag="e")
                    nc.scalar.activation(
                        ee[:, :, 0, :], pss[0][:], EXP, bias=b2ss[:, j, q : q + 1]
                    )
                    nc.scalar.activation(ee[:, :, 1, :], pss[1][:], EXP)
                    # fc0 multiply split across engines: chain A on GPSIMD,
                    # chain B on DVE, so both finish ~together and neither
                    # blocks the readout chain
                    tmp0 = p_tmp.tile([128, 2, BT], F32, tag="tmp0")
                    nc.gpsimd.tensor_mul(
                        out=tmp0[:, 0, :],
                        in0=_f32(tgt[:, 0, 0, :]),
                        in1=ee[:, 0, 0, :],
                    )
                    nc.vector.tensor_mul(
                        out=tmp0[:, 1, :],
                        in0=_f32(tgt[:, 1, 0, :]),
                        in1=ee[:, 1, 0, :],
                    )
                    # fc1 multiply on DVE with exp(b2s) folded in
                    tmp1 = p_tmp.tile([128, 2, BT], MMDT, tag="tmp1")
                    nc.vector.scalar_tensor_tensor(
                        out=tmp1[:],
                        in0=_f32(tgt[:, :, 1, :]),
                        scalar=eb2ss[:, j, q : q + 1],
                        in1=ee[:, :, 1, :],
                        op0=MULT,
                        op1=MULT,
                    )
                    tmp = {0: tmp0, 1: tmp1}
                    v = p_state.tile(
                        [128, 2, 2, BT], MMDT, tag=f"st{q}{pr % 2}", name=f"v{pr}"
                    )
                    # t matmuls AFTER the multiplies: the t psum banks are
                    # then held only across the readout, not the whole
                    # exp/multiply chain, so independent pairs can overlap
                    for fc in range(2):
                        pst[fc] = ps_st.tile(
                            [128, 2, BT], F32, tag="st", name=f"pst{fc}"
                        )
                        for ch in range(2):
                            r = 64 + 32 * ch
                            nc.tensor.matmul(
                                pst[fc][:, ch, :],
                                w2s[r : r + 32, j, q, fc * 128 : (fc + 1) * 128],
                                th[r : r + 32, ch, :],
                                tile_position=(r, 0),
                            )
                        if fc == 0:
                            # per-chain readouts chase the split GPSIMD muls
                            for ch in range(2):
                                nc.vector.scalar_tensor_tensor(
                                    out=v[:, ch, 0, :],
                                    in0=_f32(tmp0[:, ch, :]),
                                    scalar=b2ts[:, j, q, 0:1],
                                    in1=pst[0][:, ch, :],
                                    op0=ADD,
                                    op1=ADD,
                                )
                        else:
                            nc.vector.scalar_tensor_tensor(
                                out=v[:, :, 1, :],
                                in0=_f32(tmp1[:]),
                                scalar=b2ts[:, j, q, 1:2],
                                in1=pst[1][:],
                                op0=ADD,
                                op1=ADD,
                            )
                    u[q] = v

            for pr in prs:
                b0 = pr * 2 * BT
                for h in range(2):
                    for ch in range(2):
                        bch = b0 + ch * BT
                        nc.sync.dma_start(
                            out=out_d[h * S : (h + 1) * S, bch : bch + BT].rearrange(
                                "(c p) b -> p c b", p=128
                            ),
                            in_=us[pr][h][:, ch, :, :],
                        )
    nc.compile()
    return nc


_NC_CACHE = {}
TRACE = False
LAST_EXEC_NS = None
LAST_RES = None


def _get_nc(bc):
    if bc not in _NC_CACHE:
        _NC_CACHE[bc] = build_nc(bc)
    return _NC_CACHE[bc]


def kernel(x, W1, b1, W2, b2):
    global LAST_EXEC_NS
    x = np.asarray(x, np.float32)
    b = x.shape[0]
    assert b % NCORES == 0
    bc = b // NCORES
    packed = _pack_weights(W1, b1, W2, b2)
    nc = _get_nc(bc)
    in_maps = [
        {
            "x_t": np.ascontiguousarray(x[i * bc : (i + 1) * bc, :D].T),
            **packed,
        }
        for i in range(NCORES)
    ]
    res = run_bass_kernel_spmd(nc, in_maps, list(range(NCORES)), trace=TRACE)
    if getattr(res, "exec_time_ns", None):
        LAST_EXEC_NS = res.exec_time_ns
    if TRACE:
        globals()["LAST_RES"] = res
    out = np.empty((b, D + 1), np.float32)
    for i in range(NCORES):
        out[i * bc : (i + 1) * bc, :D] = res.results[i]["out"].T
    out[:, D] = x[:, D]
    return out
